# revision 14
# baseline (speedup 1.0000x reference)
"""Masked-linear kernel for trn2: out = x @ (mask.T * w) + b.

Full shapes: x (8192, 3072) f32, w (3072, 1536) f32, b (1536,) f32,
mask (1536, 3072) f32 -> out (8192, 1536) f32.

The mask is block-structured over a 3x6 grid of (512-unit x 512-input)
blocks: per unit-block two input-blocks are all-zero (skipped entirely),
two are all-ones (no mask multiply needed) and two are random 0/1
(multiplied on device).

Strategy: 8 NeuronCores as a 4 (batch) x 2 (units) grid. Each core
computes outT (768, 2048) = (w*maskT).T @ x_shard.T + b in bf16 with
full-K PSUM accumulation. K-chunks and unit-chunks are permuted on the
host so every core runs the identical module:
  - device u-chunks 0..3 ("sparse", 16 K-chunks each: dev-k 0..7 all-ones
    mask -> raw w used as lhsT, dev-k 8..15 random mask -> w*m on DVE)
  - device u-chunks 4..5 ("dense" = middle unit block, all 24 K-chunks,
    all random mask)
448 matmuls of [128x128]@[128x512] per core (the bf16 PE-array floor for
this masked pattern), ~97us of PE streaming. DMAs are issued in exact
consumption order so the PE never starves; PSUM eviction fuses the bias
add and runs on Scalar (j=0) / Vector (j=1) engines in parallel, output
in bf16 (cast back to f32 on host).
"""

import os
import sys

import numpy as np
import ml_dtypes

for _p in ("/opt/trn_rl_repo",):
    if os.path.isdir(_p) and _p not in sys.path:
        sys.path.append(_p)

import concourse.bass as bass  # noqa: E402
import concourse.mybir as mybir  # noqa: E402
import concourse.tile as tile  # noqa: E402
from concourse import bacc  # noqa: E402
from concourse.bass_utils import run_bass_kernel_spmd  # noqa: E402

BF16 = ml_dtypes.bfloat16

BATCH, IN_DIM, UNITS = 8192, 3072, 1536
BW, UW = 4, 2  # batch ways x unit ways = 8 cores
BC = BATCH // BW  # 2048 batch rows per core
UC = UNITS // UW  # 768 units per core
P = 128
BT = 512  # matmul moving free dim (one PSUM bank of f32)
NB = BC // BT  # 4
NKS = 16  # K-chunks for sparse u-groups
NKD = 24  # K-chunks for the dense u-group
N_CORES = 8

# Device K-chunk order (original k-chunk indices), per unit-shard.
# Sparse groups use dev-k 0..15: first 8 have all-ones mask, last 8 random.
# uc=0 sparse blocks (unit block 0): ds1(k0-3,p=1) stn1(k12-15,p=1)
#                                    ds2(k4-7,p=.5) stn2(k16-19,p=.5)
# uc=1 sparse blocks (unit block 2): ds3(k8-11,p=1) stn3(k20-23,p=1)
#                                    ds2(k4-7,p=.5) stn2(k16-19,p=.5)
KPERM = {
    0: [0, 1, 2, 3, 12, 13, 14, 15, 4, 5, 6, 7, 16, 17, 18, 19,
        8, 9, 10, 11, 20, 21, 22, 23],
    1: [8, 9, 10, 11, 20, 21, 22, 23, 4, 5, 6, 7, 16, 17, 18, 19,
        0, 1, 2, 3, 12, 13, 14, 15],
}
# Device u-chunk order (original 128-unit chunk indices within the shard):
# sparse chunks at device positions 0..3, dense (middle unit block) at 4..5.
UPERM = {0: [0, 1, 2, 3, 4, 5], 1: [2, 3, 4, 5, 0, 1]}

_NC_CACHE = None


def _build_module():
    nc = bacc.Bacc("TRN2", target_bir_lowering=False, debug=False)

    f32 = mybir.dt.float32
    bf16 = mybir.dt.bfloat16

    xp = nc.dram_tensor("xp", (P, NKD * BC), bf16, kind="ExternalInput")
    wg0 = nc.dram_tensor("wg0", (P, NKS * 256), bf16, kind="ExternalInput")
    wg1 = nc.dram_tensor("wg1", (P, NKS * 256), bf16, kind="ExternalInput")
    wg2 = nc.dram_tensor("wg2", (P, NKD * 256), bf16, kind="ExternalInput")
    mg0 = nc.dram_tensor("mg0", (P, 8 * 256), bf16, kind="ExternalInput")
    mg1 = nc.dram_tensor("mg1", (P, 8 * 256), bf16, kind="ExternalInput")
    mg2 = nc.dram_tensor("mg2", (P, NKD * 256), bf16, kind="ExternalInput")
    bp = nc.dram_tensor("bp", (P, 6), f32, kind="ExternalInput")
    outT = nc.dram_tensor("outT", (UC, BC), bf16, kind="ExternalOutput")

    oT3 = outT.ap().rearrange("(uo p) b -> uo p b", p=P)  # [6, 128, 2048]
    wsrc = {0: wg0, 1: wg1, 2: wg2}
    msrc = {0: mg0, 1: mg1, 2: mg2}

    with tile.TileContext(nc) as tc:
        with (
            tc.tile_pool(name="xhpool", bufs=5) as xhpool,
            tc.tile_pool(name="xpool", bufs=11) as xpool,
            tc.tile_pool(name="wk0pool", bufs=8) as wk0pool,
            tc.tile_pool(name="wpool", bufs=12) as wpool,
            tc.tile_pool(name="mpool", bufs=10) as mpool,
            tc.tile_pool(name="wmpool", bufs=10) as wmpool,
            tc.tile_pool(name="opool", bufs=4) as opool,
            tc.tile_pool(name="otfpool", bufs=4) as otfpool,
            tc.tile_pool(name="cpool", bufs=2) as cpool,
            tc.tile_pool(name="pspool", bufs=8, space="PSUM") as pspool,
        ):
            btile = cpool.tile([P, 6], f32, name="btile")

            # ---- PE warm-up: memset a tile (no DMA dependency) and issue
            # tiny matmuls so the HAM activity window starts during the
            # DMA head and real matmuls run at the full 2.4 GHz clock
            wu = cpool.tile([P, 192], bf16, name="wutile")
            nc.vector.memset(wu[:], 1.0)
            pd = pspool.tile([P, BT], f32, name="pswarm", tag="ps")
            for _ in range(55):
                nc.tensor.matmul(
                    pd[:, 0:64], wu[:, 0:P], wu[:, P:P + 64],
                    start=True, stop=True, skip_group_check=True)

            # x tiles: k0 split 512/512/1024, k1 split 1024/1024 for a fast
            # pipeline head; k>=2 in 1 MiB pair tiles for DMA efficiency
            xq = [None] * 2  # k0 b0 / b1, [P, 512]
            xh = {}          # (k, half) -> tile [P, 1024]
            xt2 = [None] * 11  # pair tiles [P, 4096] covering k 2k'+2..2k'+3
            wk0 = [None] * 8  # g0 unmasked region, per-k [P, 256]
            wt = {0: [None] * 2, 1: [None] * 4, 2: [None] * 6}
            mt = {0: [None] * 2, 1: [None] * 2, 2: [None] * 6}
            wm = {0: [None] * 2, 1: [None] * 2, 2: [None] * 6}

            def dma_xq(b):
                xq[b] = xhpool.tile([P, BT], bf16, name=f"xq{b}", tag="xh")
                nc.sync.dma_start(xq[b][:], xp.ap()[:, b * BT:(b + 1) * BT])

            def dma_xh(k, h):
                t = xhpool.tile([P, 1024], bf16, name=f"xh{k}_{h}", tag="xh")
                nc.sync.dma_start(
                    t[:], xp.ap()[:, k * BC + h * 1024: k * BC + (h + 1) * 1024])
                xh[(k, h)] = t

            def dma_xpair(i):
                # covers dev-k 2i+2 and 2i+3
                xt2[i] = xpool.tile([P, 4096], bf16, name=f"xt2_{i}", tag="xt")
                nc.sync.dma_start(
                    xt2[i][:],
                    xp.ap()[:, (2 * i + 2) * BC:(2 * i + 4) * BC])

            def dma_wk0(k):
                wk0[k] = wk0pool.tile([P, 256], bf16, name=f"wk0_{k}", tag="wk0")
                nc.sync.dma_start(
                    wk0[k][:], wg0.ap()[:, k * 256:(k + 1) * 256])

            def dma_w(g, t):
                # for g=0, t indexes the masked region (dev-k 8..15)
                off = (t + 2) * 1024 if g == 0 else t * 1024
                wt[g][t] = wpool.tile([P, 1024], bf16, name=f"wt{g}_{t}",
                                      tag="wt")
                nc.sync.dma_start(
                    wt[g][t][:], wsrc[g].ap()[:, off:off + 1024])

            def dma_m(g, t):
                # t indexes masked 4-chunk tiles (sparse groups: dev-k 8..15)
                mt[g][t] = mpool.tile([P, 1024], bf16, name=f"mt{g}_{t}",
                                      tag="mt")
                nc.sync.dma_start(
                    mt[g][t][:], msrc[g].ap()[:, t * 1024:(t + 1) * 1024])

            # ---- DMA issue in consumption order (phase g0 first) ----
            dma_xq(0)
            dma_wk0(0)
            dma_xq(1)
            dma_xh(0, 1)
            dma_wk0(1)
            dma_xh(1, 0)
            dma_xh(1, 1)
            nc.sync.dma_start(btile[:], bp.ap())
            for k in range(2, 8):
                dma_wk0(k)
                if k % 2 == 0:
                    dma_xpair((k - 2) // 2)
            dma_w(0, 0)
            dma_m(0, 0)
            dma_w(0, 1)
            dma_m(0, 1)
            for i in range(3, 7):  # x pairs k8..15
                dma_xpair(i)
            for t in range(4):
                dma_w(1, t)
            for t in range(2):
                dma_m(1, t)
            for t in range(6):
                dma_w(2, t)
            for t in range(6):
                dma_m(2, t)
            for i in range(7, 11):  # x pairs k16..23
                dma_xpair(i)

            def rhs(k, b):
                if k == 0:
                    if b < 2:
                        return xq[b][:]
                    return xh[(0, 1)][:, (b - 2) * BT:(b - 1) * BT]
                if k == 1:
                    return xh[(1, b // 2)][:, (b % 2) * BT:(b % 2 + 1) * BT]
                i, r = divmod(k - 2, 2)
                return xt2[i][:, r * BC + b * BT: r * BC + (b + 1) * BT]

            # ---- mask multiplies on DVE (all early; data-dependency paced)
            for g in (0, 1):
                for t in range(2):
                    wm[g][t] = wmpool.tile([P, 1024], bf16,
                                           name=f"wm{g}_{t}", tag="wm")
                    wsrc_t = wt[0][t] if g == 0 else wt[1][t + 2]
                    nc.vector.tensor_mul(wm[g][t][:], wsrc_t[:], mt[g][t][:])
            for t in range(6):
                wm[2][t] = wmpool.tile([P, 1024], bf16, name=f"wm2_{t}",
                                       tag="wm")
                nc.vector.tensor_mul(wm[2][t][:], wt[2][t][:], mt[2][t][:])

            def lhs_slice(g, k, j):
                """lhsT [128k x 128u] for dev-k k, u-chunk j of group g."""
                if g == 0 and k < 8:
                    return wk0[k][:, j * P:(j + 1) * P]
                if g == 1 and k < 8:
                    src = wt[1][k // 4]
                    off = (k % 4) * 256 + j * P
                elif g < 2:
                    src = wm[g][(k - 8) // 4]
                    off = ((k - 8) % 4) * 256 + j * P
                else:
                    src = wm[2][k // 4]
                    off = (k % 4) * 256 + j * P
                return src[:, off:off + P]

            def evict(ps4, uo, on_vector):
                """psum + bias -> bf16 out tile, then DMA out."""
                ot = opool.tile([P, BC], bf16, name=f"ot{uo}", tag="ot")
                bias = btile[:, uo:uo + 1]
                for b in range(NB):
                    osl = ot[:, b * BT:(b + 1) * BT]
                    if on_vector:
                        nc.vector.tensor_add(
                            osl, ps4[b][:], bias.to_broadcast((P, BT)))
                    else:
                        nc.scalar.add(osl, ps4[b][:], bias)
                nc.sync.dma_start(oT3[uo], ot[:])

            # ---- phase g0: j-interleaved k-loop (paced by x arrival) ----
            ps = [
                pspool.tile([P, BT], f32, name=f"ps0_{j}_{b}", tag="ps")
                for j in range(2) for b in range(NB)
            ]
            for k in range(NKS):
                for j in range(2):
                    lhsT = lhs_slice(0, k, j)
                    for b in range(NB):
                        nc.tensor.matmul(
                            ps[j * NB + b][:], lhsT, rhs(k, b),
                            start=(k == 0), stop=(k == NKS - 1))
            evict(ps[0:NB], 0, on_vector=False)
            evict(ps[NB:2 * NB], 1, on_vector=True)

            # ---- phases g1, g2: j-sequenced (evictions hide under MMs) ----
            for g in (1, 2):
                Kg = NKS if g < 2 else NKD
                for j in range(2):
                    uo = 2 * g + j
                    if g == 2 and j == 1:
                        # final u-chunk: b-major so each 512-batch column
                        # finishes early and its eviction + out-DMA pipeline
                        # hides under the remaining matmuls
                        bias = btile[:, uo:uo + 1]
                        for b in range(NB):
                            psb = pspool.tile([P, BT], f32,
                                              name=f"psf{b}", tag="ps")
                            for k in range(Kg):
                                nc.tensor.matmul(
                                    psb[:], lhs_slice(g, k, j), rhs(k, b),
                                    start=(k == 0), stop=(k == Kg - 1))
                            oth = otfpool.tile([P, BT], bf16,
                                               name=f"otf{b}", tag="otf")
                            if b % 2 == 0 or b == NB - 1:
                                nc.vector.tensor_add(
                                    oth[:], psb[:],
                                    bias.to_broadcast((P, BT)))
                            else:
                                nc.scalar.add(oth[:], psb[:], bias)
                            nc.sync.dma_start(
                                oT3[uo][:, b * BT:(b + 1) * BT], oth[:])
                        continue
                    ps4 = [
                        pspool.tile([P, BT], f32, name=f"ps{g}_{j}_{b}",
                                    tag="ps")
                        for b in range(NB)
                    ]
                    for k in range(Kg):
                        lhsT = lhs_slice(g, k, j)
                        for b in range(NB):
                            nc.tensor.matmul(
                                ps4[b][:], lhsT, rhs(k, b),
                                start=(k == 0), stop=(k == Kg - 1))
                    evict(ps4, uo, on_vector=(j == 1))

    nc.compile()
    return nc


def get_module():
    global _NC_CACHE
    if _NC_CACHE is None:
        _NC_CACHE = _build_module()
    return _NC_CACHE


def _check_zero_blocks(mask):
    # unit block 0 contributes nothing from ds3 / stn3; unit block 2
    # nothing from ds1 / stn1 (the kernel skips them entirely)
    assert not mask[0:512, 1024:1536].any(), "mask zero-block (ub0/ds3) dirty"
    assert not mask[0:512, 2560:3072].any(), "mask zero-block (ub0/stn3) dirty"
    assert not mask[1024:1536, 0:512].any(), "mask zero-block (ub2/ds1) dirty"
    assert not mask[1024:1536, 1536:2048].any(), "mask zero-block (ub2/stn1) dirty"


def make_in_maps(x, w, b, mask):
    x16 = x.astype(BF16)
    w16 = w.astype(np.float32)
    m32 = mask.astype(np.float32)
    _check_zero_blocks(m32)

    in_maps = []
    for c in range(N_CORES):
        bc, uc = divmod(c, UW)
        kp = KPERM[uc]
        up = UPERM[uc]

        # x: (2048, 3072) -> [128, k*2048 + col] with K-chunk perm
        xs = x16[bc * BC:(bc + 1) * BC].reshape(BC, NKD, P)
        xpk = np.ascontiguousarray(
            xs[:, kp, :].transpose(2, 1, 0)).reshape(P, NKD * BC)

        # w / mask shard, K-chunk + u-chunk reordered: (24, 128, 768)
        wl3 = np.ascontiguousarray(
            w16[:, uc * UC:(uc + 1) * UC].reshape(NKD, P, UC)[kp])
        ml3 = np.ascontiguousarray(
            m32[uc * UC:(uc + 1) * UC].T.reshape(NKD, P, UC)[kp])

        def ucols(g):
            o0, o1 = up[2 * g], up[2 * g + 1]
            return np.r_[o0 * P:(o0 + 1) * P, o1 * P:(o1 + 1) * P]

        def pack(a3):  # (K, 128, 256) -> (128, K*256)
            k = a3.shape[0]
            return np.ascontiguousarray(
                a3.transpose(1, 0, 2)).reshape(P, k * 256)

        wgs, mgs = [], []
        for g in range(3):
            cols = ucols(g)
            if g < 2:
                wk = wl3[:NKS][:, :, cols].copy()
                # dev-k 0..7 have all-ones mask by construction; multiply
                # anyway (identity in practice) so shipped lhsT is exact
                wk[:8] *= ml3[:8][:, :, cols]
                wgs.append(pack(wk).astype(BF16))
                mgs.append(pack(ml3[8:NKS][:, :, cols]).astype(BF16))
            else:
                wgs.append(pack(wl3[:, :, cols]).astype(BF16))
                mgs.append(pack(ml3[:, :, cols]).astype(BF16))

        bvec = b.astype(np.float32)[uc * UC:(uc + 1) * UC].reshape(6, P)
        bpk = np.ascontiguousarray(bvec[up].T)  # (128, 6)

        in_maps.append({
            "xp": xpk,
            "wg0": wgs[0], "wg1": wgs[1], "wg2": wgs[2],
            "mg0": mgs[0], "mg1": mgs[1], "mg2": mgs[2],
            "bp": bpk,
        })
    return in_maps


def assemble(results):
    out = np.empty((BATCH, UNITS), dtype=np.float32)
    for c in range(N_CORES):
        bc, uc = divmod(c, UW)
        up = UPERM[uc]
        oT = results[c]["outT"]  # (768, 2048) bf16, device u-chunk order
        for dc in range(6):
            oc = up[dc]
            out[bc * BC:(bc + 1) * BC,
                uc * UC + oc * P: uc * UC + (oc + 1) * P] = \
                oT[dc * P:(dc + 1) * P, :].T.astype(np.float32)
    return out


def kernel(x, w, b, mask, _trace=False, _trace_kwargs=None):
    x = np.asarray(x, dtype=np.float32)
    w = np.asarray(w, dtype=np.float32)
    b = np.asarray(b, dtype=np.float32)
    mask = np.asarray(mask, dtype=np.float32)
    nc = get_module()
    in_maps = make_in_maps(x, w, b, mask)
    res = run_bass_kernel_spmd(
        nc,
        in_maps,
        core_ids=list(range(N_CORES)),
        trace=_trace,
        **(_trace_kwargs or {}),
    )
    out = assemble(res.results)
    if _trace:
        return out, res
    return out
